# revision 19
# baseline (speedup 1.0000x reference)
# Trainium2 Bass kernel for DirectionalStockGNN (2-layer GATv2 + residual head).
#
# v2: fp16 edge pipeline.  Edges sorted by destination on host; each core owns
# N/8 contiguous dst nodes.  Per 128-edge block (grouped 4 blocks = 512 edges):
#   pm[f,t]   = xr[dst_t,f] + (ea@We)[t,f]   (one fp16 matmul, stationary
#               lhsT = [xr_win ; We], rhs = [onehot(dst) ; ea^T])
#             + xl[src_t,f]                  (per-block fp16 matmul vs ident,
#               xl rows fetched by dma_gather from fp16 table)
#   lr        = Lrelu(|att| * pm, alpha=.2)  (one ACT op; score = sum sign*lr)
#   score     = per-block matmul lhsT=lr-block rhs=sign -> pev[t,1]
#   ee        = Exp(pev)                     (ACT)
#   see[t,d]  = (iota==dst_rel_t) * ee_t     (DVE is_eq*mult, fp16)
#   pwin[d,:]+= see^T @ xg ; pden[d] += see^T @ ones   (fp16 matmuls)
# Stages are emitted with a 3-deep software pipeline so PE never waits on
# ACT/DVE results of the same group.  Only collective: AllGather of h1 (fp16).

import math
import os

import numpy as np

D = 128
DE = 4
WIN = 124
NEG = 0.2
HALF = 32768  # gather-table split row (full int16 index range)
CH = 4        # blocks per dma_gather call
NQ = 4        # SWDGE queues for gathers


# ----------------------------------------------------------------------------
# host-side schedule + blob construction
# ----------------------------------------------------------------------------
def _wrap16(idx):
    """dma_gather index layout: [128, n/16] int16, wrap-16, replicated x8."""
    n = idx.shape[0]
    assert n % 16 == 0
    iw = np.zeros((16, n // 16), np.int16)
    iw[np.arange(n) % 16, np.arange(n) // 16] = idx
    return np.tile(iw, (8, 1))  # [128, n//16]


def build_host_data(x, edge_index, edge_attr, ncores):
    N = x.shape[0]
    src0 = np.asarray(edge_index[0], dtype=np.int64)
    dst0 = np.asarray(edge_index[1], dtype=np.int64)
    ea = np.asarray(edge_attr, dtype=np.float32)

    # self loops with mean edge_attr per dst (PyG fill_value='mean')
    sums = np.zeros((N, DE), np.float32)
    np.add.at(sums, dst0, ea)
    cnts = np.bincount(dst0, minlength=N).astype(np.float32)
    loop_attr = sums / np.maximum(cnts, 1.0)[:, None]

    src = np.concatenate([src0, np.arange(N, dtype=np.int64)])
    dst = np.concatenate([dst0, np.arange(N, dtype=np.int64)])
    eaa = np.concatenate([ea, loop_attr], axis=0)

    order = np.argsort(dst, kind="stable")
    src_s = src[order]
    dst_s = dst[order]
    ea_s = eaa[order]

    NPC = N // ncores
    NW = math.ceil(NPC / WIN)
    half = min(HALF, N)

    # per-core window edge ranges (common window grid)
    starts = np.minimum(np.arange(NW + 1) * WIN, NPC)
    bounds = np.empty((ncores, NW + 1), np.int64)
    for c in range(ncores):
        bounds[c] = np.searchsorted(dst_s, c * NPC + starts)

    # per (core, window): split edges into src<HALF and src>=HALF
    nlo = np.empty((ncores, NW), np.int64)
    nhi = np.empty((ncores, NW), np.int64)
    for c in range(ncores):
        for w in range(NW):
            lo, hi = bounds[c, w], bounds[c, w + 1]
            nlo[c, w] = int((src_s[lo:hi] < half).sum())
            nhi[c, w] = int(hi - lo - nlo[c, w])
    KWLO = np.ceil(nlo.max(axis=0) / 128.0).astype(np.int64)
    KWHI = np.ceil(nhi.max(axis=0) / 128.0).astype(np.int64)
    KWLO = np.maximum(KWLO, 1)  # >=1 so every window has at least one block

    blobI = []  # int16 gather indices (wrap-16 layout)
    blobA = []  # fp16 dst_rel columns [128, kw]
    blobB = []  # fp16 [5, ew]: dst_rel row + ea^T
    for c in range(ncores):
        irecs = []
        arecs = []
        brecs = []
        for w in range(NW):
            lo, hi = bounds[c, w], bounds[c, w + 1]
            kwlo, kwhi = int(KWLO[w]), int(KWHI[w])
            kw = kwlo + kwhi
            ew = kw * 128
            base = c * NPC + w * WIN
            sw = src_s[lo:hi]
            dw = (dst_s[lo:hi] - base).astype(np.float32)
            ew_ = ea_s[lo:hi]
            mlo = sw < half
            # low half then high half, each padded to its block count
            srcp = np.zeros(ew, np.int64)
            drel = np.full(ew, 127.0, np.float32)
            eap = np.zeros((ew, DE), np.float32)
            a = int(mlo.sum())
            srcp[:a] = sw[mlo]
            drel[:a] = dw[mlo]
            eap[:a] = ew_[mlo]
            b0 = kwlo * 128
            b = int((~mlo).sum())
            srcp[b0 : b0 + b] = sw[~mlo]
            drel[b0 : b0 + b] = dw[~mlo]
            eap[b0 : b0 + b] = ew_[~mlo]
            srcp[b0 + b :] = half  # high-half pads -> rel idx 0
            ilo = _wrap16(srcp[:b0].astype(np.int16))  # [128, 8*kwlo]
            if kwhi:
                ihi = _wrap16((srcp[b0:] - half).astype(np.int16))
                irecs.append(np.concatenate([ilo, ihi], axis=1))
            else:
                irecs.append(ilo)
            arecs.append(np.ascontiguousarray(drel.reshape(kw, 128).T))
            brecs.append(np.concatenate([drel[None, :], eap.T], axis=0))
        blobI.append(np.concatenate(irecs, axis=1))  # [128, 8*sumKW]
        blobA.append(np.concatenate(arecs, axis=1))  # f32: is_equal scalar
        blobB.append(np.concatenate(brecs, axis=1).astype(np.float16))
    blobI = np.stack(blobI)
    blobA = np.stack(blobA)
    blobB = np.stack(blobB)

    KW = (KWLO + KWHI).astype(np.int64)
    koff = np.zeros(NW + 1, np.int64)  # cumulative blocks
    for w in range(NW):
        koff[w + 1] = koff[w] + int(KW[w])

    sched = dict(
        N=N, NPC=NPC, NW=NW,
        KWLO=[int(k) for k in KWLO], KWHI=[int(k) for k in KWHI],
        koff=[int(v) for v in koff], ncores=ncores, half=half,
    )
    return sched, blobI, blobA, blobB


def build_consts(ins):
    f32 = np.float32
    f16 = np.float16
    x = np.asarray(ins["x"], f32)
    consts = {}
    consts["xT"] = np.ascontiguousarray(x.T.astype(f16))  # [128, N]
    for li in (1, 2):
        consts[f"wl{li}"] = np.asarray(ins[f"W{li}l"], f32).astype(f16)
        consts[f"wr{li}"] = np.asarray(ins[f"W{li}r"], f32).astype(f16)
        consts[f"we{li}"] = np.asarray(ins[f"W{li}e"], f32).astype(f16)  # [4,128]
        a = np.asarray(ins[f"att{li}"], f32)
        consts[f"attabs{li}"] = np.ascontiguousarray(np.abs(a)[:, None])
        consts[f"sgn{li}"] = np.ascontiguousarray(np.sign(a)[:, None]).astype(f16)
        b = np.asarray(ins[f"b{li}"], f32)
        consts[f"bb{li}"] = np.ascontiguousarray(np.tile(b[None, :], (D, 1)))
    consts["wfc"] = np.asarray(ins["Wfc"], f32).reshape(D, 1).astype(f16)
    consts["iota_bc"] = np.ascontiguousarray(
        np.tile(np.arange(WIN, dtype=f32)[None, :], (D, 1))
    ).astype(f16)
    consts["iotacol"] = np.arange(WIN, dtype=f32)[:, None].copy()
    consts["ones1"] = np.ones((1, WIN), f16)
    consts["onec"] = np.ones((D, 1), f16)
    consts["ident"] = np.eye(D, dtype=f16)
    consts["identw"] = np.eye(D, dtype=f32)
    return consts


# ----------------------------------------------------------------------------
# bass program
# ----------------------------------------------------------------------------
def build_program(sched, bfc_val, has_bias=None):
    import concourse.bacc as bacc
    import concourse.mybir as mybir
    import concourse.tile as tile

    f32 = mybir.dt.float32
    f16 = mybir.dt.float16
    i16 = mybir.dt.int16
    Alu = mybir.AluOpType
    Act = mybir.ActivationFunctionType

    ncores = sched["ncores"]
    N, NPC, NW = sched["N"], sched["NPC"], sched["NW"]
    KWLO, KWHI = sched["KWLO"], sched["KWHI"]
    koff = sched["koff"]
    half = sched["half"]
    KW = [KWLO[w] + KWHI[w] for w in range(NW)]
    KWMAX = max(KW)
    EWMAX = KWMAX * 128
    HT = NW * WIN

    if has_bias is None:
        has_bias = {1: True, 2: True}
    nc = bacc.Bacc(
        "TRN2", target_bir_lowering=False, debug=False,
        enable_asserts=False, num_devices=ncores,
        num_swdge_queues=NQ,
    )

    # ---- I/O ----
    t_xT = nc.dram_tensor("xT", [D, N], f16, kind="ExternalInput")
    t_xT_own = nc.dram_tensor("xT_own", [D, NPC], f16, kind="ExternalInput")
    KTOT = koff[NW]
    t_blobI = nc.dram_tensor("blobI", [128, 8 * KTOT], i16, kind="ExternalInput")
    t_blobA = nc.dram_tensor("blobA", [128, KTOT], f32, kind="ExternalInput")
    t_blobB = nc.dram_tensor("blobB", [5, 128 * KTOT], f16, kind="ExternalInput")
    cshapes = dict(
        wl1=([D, D], f16), wr1=([D, D], f16), wl2=([D, D], f16),
        wr2=([D, D], f16), we1=([DE, D], f16), we2=([DE, D], f16),
        attabs1=([D, 1], f32), sgn1=([D, 1], f16),
        attabs2=([D, 1], f32), sgn2=([D, 1], f16),
        bb1=([D, D], f32), bb2=([D, D], f32), wfc=([D, 1], f16),
        iota_bc=([D, WIN], f16), iotacol=([WIN, 1], f32),
        ones1=([1, WIN], f16), onec=([D, 1], f16), ident=([D, D], f16),
        identw=([D, D], f32),
    )
    t_c = {k: nc.dram_tensor(k, sh, dt, kind="ExternalInput")
           for k, (sh, dt) in cshapes.items()}
    t_y = nc.dram_tensor("y", [NPC, 1], f32, kind="ExternalOutput")

    # ---- DRAM internals ----
    t_tab1 = nc.dram_tensor("tab1", [N, D], f16, kind="Internal")
    t_tab2 = nc.dram_tensor("tab2", [N, D], f16, kind="Internal")
    CW = 2176  # AllGather chunk width (multiple of 128)
    NCH = (NPC + CW - 1) // CW
    t_h1T_own = nc.dram_tensor("h1T_own", [NCH, D, CW], f16, kind="Internal")
    t_rhs = nc.dram_tensor("rhsT", [128, 128 * KTOT], f16, kind="Internal")
    t_h1T_all = nc.dram_tensor(
        "h1T_all", [NCH, ncores, D, CW], f16, kind="Internal",
        addr_space=("Shared" if ncores > 1 else "Local"),
    )

    with tile.TileContext(nc) as tc:
        with (
            tc.tile_pool(name="cpool", bufs=1) as cpool,
            tc.tile_pool(name="sp", bufs=3) as sp,
            tc.tile_pool(name="sp2", bufs=4) as sp2,
            tc.tile_pool(name="pm", bufs=2, space="PSUM") as pmp,
            tc.tile_pool(name="pbc", bufs=2, space="PSUM") as pbcp,
            tc.tile_pool(name="pev", bufs=1, space="PSUM") as pevp,
            tc.tile_pool(name="pwin", bufs=2, space="PSUM") as pwinp,
        ):
            # ---- load consts ----
            C = {}
            for k, (sh, dt) in cshapes.items():
                C[k] = cpool.tile(sh, dt, tag=f"c_{k}", name=f"c_{k}")
                nc.sync.dma_start(out=C[k][:], in_=t_c[k][:])

            lhsT_sb = cpool.tile([D, NW, D], f16, tag="lhsT_sb", name="lhsT_sb")
            hT_res = cpool.tile([D, HT], f16, tag="hT_res", name="hT_res")
            y_sb = cpool.tile([1, HT], f32, tag="y_sb", name="y_sb")

            qn_state = [0]

            cpy_state = [0]

            def psum_copy(out, in_, force=None):
                # alternate PSUM->SBUF copies between ACT and DVE
                cpy_state[0] += 1
                if force == "act" or (force is None and cpy_state[0] % 2 == 0):
                    nc.scalar.copy(out=out, in_=in_)
                else:
                    nc.vector.tensor_scalar(
                        out=out, in0=in_, scalar1=1.0, scalar2=None, op0=Alu.mult
                    )

            def dense_table(layer, t_tab, agc=None):
                wl = C[f"wl{layer}"]
                if layer == 1:
                    srcs = [(None, 0, N)]
                else:
                    srcs = [
                        (c8, c8 * NPC + off, (k, cn))
                        for k, off, cn in agc
                        for c8 in range(ncores)
                    ]
                def _rows(spec):
                    return spec if isinstance(spec, int) else spec[1]

                nchunks = sum((_rows(sp_) + 511) // 512 for _, _, sp_ in srcs)
                ci = 0
                widx = [0]

                def drain_rhs(frac):
                    if layer != 1:
                        return
                    while widx[0] < min(NW, int(frac * NW + 1e-9)):
                        build_rhs_window(widx[0])
                        widx[0] += 1

                for c8, gbase, spec in srcs:
                    kch = None if isinstance(spec, int) else spec[0]
                    nrows = _rows(spec)
                    for rr in range(0, nrows, 512):
                        r0 = rr
                        ci += 1
                        drain_rhs(ci / nchunks)
                        rn = min(512, nrows - rr)
                        nb = (rn + 127) // 128
                        xt_t = sp.tile([D, 512], f16, tag="xt_t", name="xt_t")
                        if layer == 1:
                            nc.sync.dma_start(out=xt_t[:, :rn], in_=t_xT[:, r0 : r0 + rn])
                        else:
                            nc.sync.dma_start(
                                out=xt_t[:, :rn],
                                in_=t_h1T_all[kch, c8, :, r0 : r0 + rn],
                            )
                        stg = sp.tile([D, 4, D], f16, tag="stg", name="stg")
                        for k in range(nb):
                            rk = min(128, rn - k * 128)
                            ps = pbcp.tile([D, 512], f32, tag="pbc", name="ps")
                            nc.tensor.matmul(
                                out=ps[:rk, :128], lhsT=xt_t[:, k * 128 : k * 128 + rk],
                                rhs=wl[:, :], start=True, stop=True,
                            )
                            psum_copy(stg[:rk, k, :], ps[:rk, :128],
                                      force=("act" if layer == 1 else None))
                        nfull = rn // 128
                        if nfull:
                            out_ap = t_tab[
                                gbase + rr : gbase + rr + nfull * 128, :
                            ].rearrange("(cb p) f -> p cb f", p=128)
                            nc.scalar.dma_start(out=out_ap, in_=stg[:, 0:nfull, :])
                        if rn % 128:
                            rk = rn % 128
                            nc.scalar.dma_start(
                                out=t_tab[gbase + rr + nfull * 128 : gbase + rr + rn, :],
                                in_=stg[:rk, nfull, :],
                            )

            def dense_xr(layer):
                wr = C[f"wr{layer}"]
                nc.vector.memset(lhsT_sb[:, :, :], 0.0)
                xw = {}

                def load_chunk(ci):
                    if layer != 1 or ci * 4 >= NW or ci in xw:
                        return
                    w0 = ci * 4
                    cols0 = w0 * WIN
                    ncols = min(NPC, cols0 + 4 * WIN) - cols0
                    xt_t = sp.tile([D, 4 * WIN], f16, tag="xt_w", name="xt_w")
                    nc.sync.dma_start(
                        out=xt_t[:, :ncols], in_=t_xT_own[:, cols0 : cols0 + ncols]
                    )
                    xw[ci] = xt_t

                load_chunk(0)
                load_chunk(1)
                for w in range(NW):
                    wn = min(WIN, NPC - w * WIN)
                    if w % 4 == 0:
                        load_chunk(w // 4 + 2)
                    if layer == 1:
                        lhs = xw[w // 4][:, (w % 4) * WIN : (w % 4) * WIN + wn]
                    else:
                        lhs = hT_res[:, w * WIN : w * WIN + wn]
                    ps = pbcp.tile([D, 512], f32, tag="pbc", name="psx")
                    nc.tensor.matmul(
                        out=ps[:wn, :128], lhsT=lhs, rhs=wr[:, :],
                        start=True, stop=True,
                    )
                    psum_copy(lhsT_sb[:wn, w, 0:128], ps[:wn, :128])
                    nc.scalar.dma_start(
                        out=lhsT_sb[124:128, w, 0:128], in_=t_c[f"we{layer}"][:, :]
                    )

            def build_rhs_window(w):
                kwlo, kwhi = KWLO[w], KWHI[w]
                kw = kwlo + kwhi
                ew = kw * 128
                ko = koff[w]
                dstrow = sp.tile([1, EWMAX], f16, tag="bdst", name="bdst")
                nc.sync.dma_start(
                    out=dstrow[:, :ew], in_=t_blobB[0:1, 128 * ko : 128 * ko + ew]
                )
                rhs_t = sp.tile([D, EWMAX], f16, tag="brhs", name="brhs")
                nc.sync.dma_start(
                    out=rhs_t[124:128, :ew],
                    in_=t_blobB[1:5, 128 * ko : 128 * ko + ew],
                )
                for t0 in range(0, kw, 4):
                    nb = min(4, kw - t0)
                    T = nb * 128
                    c0 = t0 * 128
                    pbc_t = pbcp.tile([D, 512], f32, tag="pbc", name="pbc")
                    nc.tensor.matmul(
                        out=pbc_t[0:WIN, :T], lhsT=C["ones1"][:, :],
                        rhs=dstrow[:, c0 : c0 + T], start=True, stop=True,
                    )
                    nc.vector.tensor_scalar(
                        out=rhs_t[0:WIN, c0 : c0 + T], in0=pbc_t[0:WIN, :T],
                        scalar1=C["iotacol"][:, :], scalar2=None,
                        op0=Alu.is_equal,
                    )
                nc.scalar.dma_start(
                    out=t_rhs[:, 128 * ko : 128 * ko + ew], in_=rhs_t[:, :ew]
                )

            # ------------- edge pass: 3-deep software pipeline ---------------
            def edge_pass(layer, t_tab):
                attabs = C[f"attabs{layer}"]
                sgn = C[f"sgn{layer}"]
                items = []  # (w, g, t0, nb)
                for w in range(NW):
                    kw = KW[w]
                    ngr = (kw + 3) // 4
                    for g in range(ngr):
                        t0 = g * 4
                        items.append((w, g, t0, min(4, kw - t0)))
                wstate = {}
                gstate = {}

                def loads(w):
                    kwlo, kwhi = KWLO[w], KWHI[w]
                    kw = kwlo + kwhi
                    ew = kw * 128
                    ko = koff[w]
                    it = sp2.tile([D, 8 * KWMAX], i16, tag="it", name="it")
                    nc.sync.dma_start(
                        out=it[:, : 8 * kw],
                        in_=t_blobI[:, 8 * ko : 8 * ko + 8 * kw],
                    )
                    at = sp2.tile([D, KWMAX], f32, tag="at", name="at")
                    nc.sync.dma_start(out=at[:, :kw], in_=t_blobA[:, ko : ko + kw])
                    rhs_t = sp2.tile([D, EWMAX], f16, tag="rhs_t", name="rhs_t")
                    nc.sync.dma_start(
                        out=rhs_t[:, :ew],
                        in_=t_rhs[:, 128 * ko : 128 * ko + ew],
                    )
                    xg = sp2.tile([D, KWMAX + 1, D], f16, tag="xg", name="xg")
                    nc.vector.memset(xg[:, KWMAX, :], 1.0)

                    def do_gathers(base_blk, nblk, tab_ap, icol0):
                        for g0 in range(0, nblk, CH):
                            gn = min(CH, nblk - g0)
                            nc.gpsimd.dma_gather(
                                out_ap=xg[:, base_blk + g0 : base_blk + g0 + gn, :],
                                in_ap=tab_ap,
                                idxs_ap=it[:, icol0 + 8 * g0 : icol0 + 8 * (g0 + gn)],
                                num_idxs=gn * 128,
                                num_idxs_reg=gn * 128,
                                elem_size=D,
                                queue_num=qn_state[0],
                            )
                            qn_state[0] = (qn_state[0] + 1) % NQ

                    do_gathers(0, kwlo, t_tab[0:half, :], 0)
                    if kwhi:
                        do_gathers(kwlo, kwhi, t_tab[half:N, :], 8 * kwlo)
                    wstate[w] = dict(at=at, rhs_t=rhs_t, xg=xg)

                def stage_a(w, g, t0, nb):
                    # rhs_t comes prebuilt from t_rhs; keep loads 2 windows ahead
                    if g == 0 and w + 2 < NW:
                        loads(w + 2)

                def stage_b(w, g, t0, nb):
                    # m in PSUM: stationary matmul + gathered-x transposes
                    ws = wstate[w]
                    T = nb * 128
                    c0 = t0 * 128
                    pm = pmp.tile([D, 512], f32, tag="pm", name="pm")
                    nc.tensor.matmul(
                        out=pm[:, :T], lhsT=lhsT_sb[:, w, :],
                        rhs=ws["rhs_t"][:, c0 : c0 + T], start=True, stop=False,
                    )
                    for cb in range(nb):
                        nc.tensor.matmul(
                            out=pm[:, cb * 128 : (cb + 1) * 128],
                            lhsT=ws["xg"][:, t0 + cb, :],
                            rhs=C["ident"][:, :],
                            start=False, stop=(cb == nb - 1),
                        )
                    gstate[(w, g)] = dict(pm=pm)

                def stage_c1(w, g, t0, nb):
                    # leaky-relu -> per-block score matmuls -> exp
                    gs = gstate[(w, g)]
                    T = nb * 128
                    lr = sp.tile([D, 512], f16, tag="lr", name="lr")
                    nc.scalar.activation(
                        out=lr[:, :T], in_=gs["pm"][:, :T], func=Act.Prelu,
                        scale=attabs[:, :], alpha=NEG,
                    )
                    pev = pevp.tile([D, 4], f32, tag="pev", name="pev")
                    for cb in range(nb):
                        nc.tensor.matmul(
                            out=pev[:, cb : cb + 1],
                            lhsT=lr[:, cb * 128 : (cb + 1) * 128],
                            rhs=sgn[:, :], start=True, stop=True,
                        )
                    ee = sp.tile([D, 4], f32, tag="ee", name="ee")
                    nc.scalar.activation(out=ee[:, :nb], in_=pev[:, :nb], func=Act.Exp)
                    gs["ee"] = ee

                def stage_c2(w, g, t0, nb):
                    # scatter exp into onehot columns (see)
                    ws = wstate[w]
                    gs = gstate[(w, g)]
                    ee = gs.pop("ee")
                    see = sp.tile([D, 4 * WIN], f16, tag="see", name="see")
                    for cb in range(nb):
                        nc.vector.tensor_scalar(
                            out=see[:, cb * WIN : (cb + 1) * WIN],
                            in0=C["iota_bc"][:, :],
                            scalar1=ws["at"][:, t0 + cb : t0 + cb + 1],
                            scalar2=ee[:, cb : cb + 1],
                            op0=Alu.is_equal, op1=Alu.mult,
                        )
                    gs["see"] = see

                def stage_d(w, g, t0, nb):
                    # weighted aggregation + denominator
                    ws = wstate[w]
                    gs = gstate.pop((w, g))
                    kw = KW[w]
                    see = gs["see"]
                    if g == 0:
                        wstate[w]["pwin"] = pwinp.tile(
                            [WIN, 256], f32, tag="pwin", name="pwin"
                        )
                    pwin = wstate[w]["pwin"]
                    for cb in range(nb):
                        glob_b = t0 + cb
                        step = KWMAX - (t0 + cb)
                        nc.tensor.matmul(
                            out=pwin[0:WIN, 0:256],
                            lhsT=see[:, cb * WIN : (cb + 1) * WIN],
                            rhs=ws["xg"][:, t0 + cb :: step, :],
                            start=(glob_b == 0), stop=(glob_b == kw - 1),
                        )
                    if t0 + nb == kw:
                        epilogue(w)

                def epilogue(w):
                    ws = wstate.pop(w)
                    pwin = ws["pwin"]
                    wn = min(WIN, NPC - w * WIN)
                    den = sp.tile([WIN, 1], f32, tag="den", name="den")
                    nc.vector.tensor_scalar(
                        out=den[:, :], in0=pwin[0:WIN, 128:129],
                        scalar1=1e-30, scalar2=None, op0=Alu.max,
                    )
                    rec = sp.tile([WIN, 1], f32, tag="rec", name="rec")
                    nc.vector.reciprocal(out=rec[:, :], in_=den[:, :])
                    hw_ = sp.tile([WIN, D], f32, tag="hw", name="hw_")
                    nc.scalar.activation(
                        out=hw_[:, :], in_=pwin[0:WIN, 0:128], func=Act.Copy,
                        scale=rec[:, :],
                    )
                    if has_bias[layer]:
                        nc.vector.tensor_tensor(
                            out=hw_[:, :], in0=hw_[:, :],
                            in1=C[f"bb{layer}"][0:WIN, :], op=Alu.add,
                        )
                    # ELU: h - min(h,0) + exp(min(h,0)) - 1
                    tmin = sp.tile([WIN, D], f32, tag="tmin", name="tmin")
                    nc.vector.tensor_scalar(
                        out=tmin[:, :], in0=hw_[:, :], scalar1=0.0, scalar2=None,
                        op0=Alu.min,
                    )
                    uexp = sp.tile([WIN, D], f32, tag="uexp", name="uexp")
                    nc.scalar.activation(out=uexp[:, :], in_=tmin[:, :], func=Act.Exp)
                    nc.vector.tensor_tensor(
                        out=hw_[:, :], in0=hw_[:, :], in1=tmin[:, :], op=Alu.subtract
                    )
                    nc.vector.tensor_scalar(
                        out=uexp[:, :], in0=uexp[:, :], scalar1=-1.0, scalar2=None,
                        op0=Alu.add,
                    )
                    nc.vector.tensor_tensor(
                        out=hw_[:, :], in0=hw_[:, :], in1=uexp[:, :], op=Alu.add
                    )
                    # transpose h window -> [128f, wn]
                    pt = pbcp.tile([D, 512], f32, tag="pbc", name="pt")
                    nc.tensor.matmul(
                        out=pt[:, 0:WIN], lhsT=hw_[:, :], rhs=C["identw"][0:WIN, 0:WIN],
                        is_transpose=True, start=True, stop=True,
                    )
                    if layer == 1:
                        nc.scalar.copy(
                            out=hT_res[:, w * WIN : w * WIN + WIN], in_=pt[:, 0:WIN]
                        )
                    else:
                        h2t = sp.tile([D, WIN], f16, tag="h2t", name="h2t")
                        nc.scalar.copy(out=h2t[:, :], in_=pt[:, 0:WIN])
                        xt_f = sp.tile([D, WIN], f16, tag="xt_fin", name="xt_f")
                        nc.sync.dma_start(
                            out=xt_f[:, :wn], in_=t_xT_own[:, w * WIN : w * WIN + wn]
                        )
                        nc.vector.tensor_tensor(
                            out=h2t[:, :wn], in0=h2t[:, :wn], in1=xt_f[:, :wn],
                            op=Alu.add,
                        )
                        py = pbcp.tile([D, 512], f32, tag="pbc", name="py")
                        nc.tensor.matmul(
                            out=py[0:1, :wn], lhsT=C["wfc"][:, :], rhs=h2t[:, :wn],
                            start=True, stop=True,
                        )
                        nc.scalar.activation(
                            out=y_sb[:, w * WIN : w * WIN + wn], in_=py[0:1, :wn],
                            func=Act.Copy, bias=float(bfc_val),
                        )

                for w0 in range(min(2, NW)):
                    loads(w0)
                ni = len(items)
                for i in range(ni + 5):
                    if 0 <= i - 5 < ni:
                        stage_d(*items[i - 5])
                    if 0 <= i - 4 < ni:
                        stage_c2(*items[i - 4])
                    if 0 <= i - 2 < ni:
                        stage_c1(*items[i - 2])
                    if 0 <= i - 1 < ni:
                        stage_b(*items[i - 1])
                    if i < ni:
                        stage_a(*items[i])

            # ---------------- phases (GNN_MAXPHASE truncates for bisect) ----
            maxphase = int(os.environ.get("GNN_MAXPHASE", "6"))

            def body():
                if maxphase < 6:
                    nc.vector.memset(y_sb[:, :], 0.0)
                dense_table(1, t_tab1)
                if maxphase >= 1:
                    dense_xr(1)
                if maxphase >= 2:
                    edge_pass(1, t_tab1)
                    for k in range(NCH):
                        off = k * CW
                        cn = min(CW, NPC - off)
                        nc.sync.dma_start(
                            out=t_h1T_own[k, :, 0:cn],
                            in_=hT_res[:, off : off + cn],
                        )
                agc = [(k, k * CW, min(CW, NPC - k * CW)) for k in range(NCH)]
                if maxphase >= 3:
                    for k, off, cn in agc:
                        if ncores > 1:
                            nc.gpsimd.collective_compute(
                                "AllGather",
                                mybir.AluOpType.bypass,
                                replica_groups=[list(range(ncores))],
                                ins=[t_h1T_own[k, :, :]],
                                outs=[t_h1T_all[k, :, :, :]],
                            )
                        else:
                            nc.sync.dma_start(
                                out=t_h1T_all[0, 0, :, :], in_=t_h1T_own[0, :, :]
                            )
                if maxphase >= 5:
                    dense_xr(2)
                if maxphase >= 4:
                    dense_table(2, t_tab2, agc)
                if maxphase >= 6:
                    edge_pass(2, t_tab2)

            body()
            nc.sync.dma_start(out=t_y[:, 0], in_=y_sb[0:1, 0:NPC])

    nc.compile()
    return nc


# ----------------------------------------------------------------------------
# entry points
# ----------------------------------------------------------------------------
def prepare(inputs, ncores=8):
    x = np.asarray(inputs["x"], np.float32)
    sched, blobI, blobA, blobB = build_host_data(
        x, inputs["edge_index"], inputs["edge_attr"], ncores
    )
    consts = build_consts(inputs)
    bfc_val = float(np.asarray(inputs["bfc"]).reshape(-1)[0])
    has_bias = {
        li: bool(np.any(np.asarray(inputs[f"b{li}"], np.float32)))
        for li in (1, 2)
    }
    nc = build_program(sched, bfc_val, has_bias)
    NPC = sched["NPC"]
    in_maps = []
    for c in range(ncores):
        m = dict(consts)
        m["xT_own"] = np.ascontiguousarray(consts["xT"][:, c * NPC : (c + 1) * NPC])
        m["blobI"] = np.ascontiguousarray(blobI[c])
        m["blobA"] = np.ascontiguousarray(blobA[c])
        m["blobB"] = np.ascontiguousarray(blobB[c])
        in_maps.append(m)
    return nc, in_maps, sched


def kernel(**inputs) -> np.ndarray:
    ncores = 8
    nc, in_maps, sched = prepare(inputs, ncores)
    from concourse.bass_utils import run_bass_kernel_spmd

    res = run_bass_kernel_spmd(nc, in_maps, core_ids=list(range(ncores)))
    y = np.concatenate([res.results[c]["y"] for c in range(ncores)], axis=0)
    return y.astype(np.float32)
